# revision 9
# baseline (speedup 1.0000x reference)
"""EntNet scan kernel v3: transposed-state layout, 8 TRN2 cores (SPMD).

Same problem split as v2 (M sharded across 8 cores, 256 slots each), but
the scan state inside a core is kept TRANSPOSED per 128-slot block:
  Ht[b]  : [m=128, d=128]  (slots on partitions)
  H_sb   : [d, 256]        (natural layout, for the matmuls)

Per step per block:
  Ht = u' * rs            (tensor_scalar, rs is a [m,1] per-partition scalar)
  psum_H = transpose(Ht)  (PE) -> copy to H_sb half (DVE)
  gate:  psum_g[:,b] = W0_blk^T s (early) + H_blk^T s   -> sigmoid -> G [m,1]
  nH:    psum_xh = Y@W0_blk (early) + X@H_blk -> tanh(+Zs bias) -> nh [d,128]
  psum_nhT = transpose(nh)
  u'  = (psum_nhT * G) + Ht        (scalar_tensor_tensor, one op)
  sq  = (u' * 0.5) * u', accum_out -> ss = 0.5*colsumsq  (one op)
  rs  = rsqrt(2*ss) via magic seed + fused Newton steps   ([m,1] tiny ops)

No broadcast matmuls, no rs-transpose chain; gate & norm scales ride the
partition-scalar operand of DVE ops. The two blocks are independent
chains that pipeline across engines.
"""

import os
import sys

if "/opt/trn_rl_repo" not in sys.path:
    sys.path.insert(0, "/opt/trn_rl_repo")

import numpy as np

D = 128
PAD = 64
M = 2048
NF = 4096
N_CORES = 8
M_LOC = M // N_CORES  # 256
MB = 128              # slots per block
NBLK = M_LOC // MB    # 2
T_LOC = NF // N_CORES  # 512
UNROLL = int(os.environ.get('UNROLL', '8'))
MAGIC_HALF = 0x5F3759DF - 0x400000
NEWTON_ITERS = int(os.environ.get('NEWTON_ITERS', '1'))
STAGGERED = os.environ.get('STAGGERED', '1') == '1'

_CACHE = {}


def _build_nc(nf_steps):
    import concourse.bass as bass
    import concourse.mybir as mybir
    import concourse.tile as tile
    import concourse.bacc as bacc
    from concourse.dve_ops import RECIPROCAL_APPROX_NR

    F32 = mybir.dt.float32
    F32R = mybir.dt.float32r
    I32 = mybir.dt.int32
    AT = mybir.ActivationFunctionType
    OP = mybir.AluOpType
    AX = mybir.AxisListType
    GROUP = [list(range(N_CORES))]

    MMD = F32R
    nc = bacc.Bacc("TRN2", target_bir_lowering=False, debug=False,
                   num_devices=N_CORES)

    e_in = nc.dram_tensor("e", [T_LOC, D, PAD], F32, kind="ExternalInput")
    f_in = nc.dram_tensor("f", [D, PAD], F32, kind="ExternalInput")
    q_in = nc.dram_tensor("qq", [D, PAD], F32, kind="ExternalInput")
    xt_in = nc.dram_tensor("xt", [D, D], F32, kind="ExternalInput")
    yt_in = nc.dram_tensor("yt", [D, D], F32, kind="ExternalInput")
    zt_in = nc.dram_tensor("zt", [D, D], F32, kind="ExternalInput")
    kt_in = nc.dram_tensor("kt", [D, D], F32, kind="ExternalInput")
    rnt_in = nc.dram_tensor("rnt", [D, D], F32, kind="ExternalInput")
    w0_in = nc.dram_tensor("w0", [D, M_LOC], F32, kind="ExternalInput")
    h0t_in = nc.dram_tensor("h0t", [NBLK, MB, D], F32, kind="ExternalInput")
    id_in = nc.dram_tensor("ident", [D, D], F32, kind="ExternalInput")
    ans_out = nc.dram_tensor("ans", [D, 1], F32, kind="ExternalOutput")
    hout = nc.dram_tensor("hout", [D, M_LOC], F32, kind="ExternalOutput")

    with tile.TileContext(nc) as tc:
        with (
            tc.tile_pool(name="const", bufs=1) as cp,
            tc.tile_pool(name="state", bufs=1) as st,
            tc.tile_pool(name="dram", bufs=1, space="DRAM") as dram,
        ):
            # ---- constants ----
            f_sb = cp.tile([D, PAD], F32)
            q_sb = cp.tile([D, PAD], F32)
            ident = cp.tile([D, D], F32)
            w0 = cp.tile([D, M_LOC], F32)
            nc.sync.dma_start(f_sb[:], f_in[:])
            nc.sync.dma_start(q_sb[:], q_in[:])
            nc.sync.dma_start(ident[:], id_in[:])
            nc.sync.dma_start(w0[:], w0_in[:])

            xt_f = cp.tile([D, D], F32)
            yt_f = cp.tile([D, D], F32)
            zt_f = cp.tile([D, D], F32)
            nc.sync.dma_start(xt_f[:], xt_in[:])
            nc.sync.dma_start(yt_f[:], yt_in[:])
            nc.sync.dma_start(zt_f[:], zt_in[:])

            # q column = rowsum(F*Q)
            fq = cp.tile([D, PAD], F32)
            nc.vector.tensor_tensor(fq[:], f_sb[:], q_sb[:], OP.mult)
            q_col = cp.tile([D, 1], F32)
            nc.vector.tensor_reduce(q_col[:], fq[:], AX.X, OP.add)
            ones_row_f = cp.tile([1, D], F32)
            nc.gpsimd.memset(ones_row_f[:], 1.0)

            # transposed state per block (u' doubles as H^T pre-scale)
            up = [st.tile([MB, D], F32, tag=f"up{b}", name=f"up{b}") for b in range(NBLK)]
            rs = [st.tile([MB, 1], F32, tag=f"rs{b}", name=f"rs{b}") for b in range(NBLK)]
            H_sb = st.tile([D, M_LOC], F32)
            for b in range(NBLK):
                nc.sync.dma_start(up[b][:], h0t_in[b])
                nc.gpsimd.memset(rs[b][:], 1.0)

            # ---- phase 1: S_loc from this core's E shard ----
            S_loc = st.tile([D, T_LOC], F32)
            TC = 64
            with tc.tile_pool(name="ephase", bufs=2) as ep:
                for c0 in range(0, T_LOC, TC):
                    e_sb = ep.tile([D, TC, PAD], F32)
                    e_ap = bass.AP(
                        tensor=e_in[:].tensor,
                        offset=c0 * D * PAD,
                        ap=[[PAD, D], [D * PAD, TC], [1, PAD]],
                    )
                    nc.sync.dma_start(e_sb[:], e_ap)
                    fe = ep.tile([D, TC, PAD], F32)
                    fb = f_sb[:]
                    f_bcast = bass.AP(
                        tensor=fb.tensor, offset=fb.offset,
                        ap=[[fb.ap[0][0], D], [0, TC], [1, PAD]],
                    )
                    nc.vector.tensor_tensor(fe[:], e_sb[:], f_bcast, OP.mult)
                    nc.vector.tensor_reduce(
                        S_loc[:, c0 : c0 + TC], fe[:], AX.X, OP.add
                    )

            # ---- phase 2: AllGather S ----
            s_bounce = dram.tile([D, T_LOC], F32)
            s_all = dram.tile([N_CORES, D, T_LOC], F32)
            nc.sync.dma_start(s_bounce[:], S_loc[:])
            nc.gpsimd.collective_compute(
                "AllGather", OP.bypass, replica_groups=GROUP,
                ins=[s_bounce[:]], outs=[s_all[:]],
            )
            S_f = st.tile([D, NF], F32)
            for r in range(N_CORES):
                nc.sync.dma_start(
                    S_f[:, r * T_LOC : (r + 1) * T_LOC], s_all[r]
                )

            # ---- phase 3: ZS = Z @ S ----
            ZS = st.tile([D, NF], F32)
            with tc.tile_pool(name="zsp", bufs=2, space="PSUM") as zp:
                for j in range(0, NF, 512):
                    ps = zp.tile([D, 512], F32)
                    nc.tensor.matmul(ps[:], zt_f[:], S_f[:, j : j + 512])
                    nc.vector.tensor_copy(ZS[:, j : j + 512], ps[:])

            # ---- phase 4: the scan ----
            with (
                tc.tile_pool(name="loop", bufs=2) as lp,
                tc.tile_pool(name="ps_h", bufs=1, space="PSUM") as ph,
                tc.tile_pool(name="ps_g", bufs=1, space="PSUM") as pg,
                tc.tile_pool(name="ps_x", bufs=1, space="PSUM") as px,
                tc.tile_pool(name="ps_n", bufs=1, space="PSUM") as pnt,
            ):
                def block_step(t_idx, b, s_slot, zs_slot):
                    hs = H_sb[:, b * MB : (b + 1) * MB]
                    w0b = w0[:, b * MB : (b + 1) * MB]

                    # normalize into transposed state
                    Ht = lp.tile([MB, D], F32, tag=f"Ht{b}")
                    nc.vector.tensor_scalar(
                        Ht[:], up[b][:], rs[b][:], None, OP.mult
                    )
                    # refresh natural-layout H for the matmuls
                    psum_H = ph.tile([D, MB], F32, tag=f"pH{b}")
                    nc.tensor.matmul(
                        psum_H[:], Ht[:], ident[:], is_transpose=True
                    )
                    if b == 1:
                        nc.scalar.activation(hs, psum_H[:], AT.Copy)
                    else:
                        nc.vector.tensor_copy(hs, psum_H[:])

                    # gate column: W0^T s (early) + H^T s, then sigmoid
                    gcol = pg.tile([MB, 1], F32, tag=f"pg{b}")
                    nc.tensor.matmul(
                        gcol[:], w0b, s_slot[:], start=True, stop=False
                    )
                    nc.tensor.matmul(
                        gcol[:], hs, s_slot[:], start=False, stop=True
                    )
                    G = lp.tile([MB, 1], F32, tag=f"G{b}")
                    nc.scalar.activation(G[:], gcol[:], AT.Sigmoid)

                    # nH: Y@W0 (early) + X@H, tanh with Zs bias
                    psum_xh = px.tile([D, MB], F32, tag=f"pxh{b}")
                    nc.tensor.matmul(
                        psum_xh[:], yt_f[:], w0b, start=True, stop=False
                    )
                    nc.tensor.matmul(
                        psum_xh[:], xt_f[:], hs, start=False, stop=True
                    )
                    nh = lp.tile([D, MB], F32, tag=f"nh{b}")
                    nc.scalar.activation(
                        nh[:], psum_xh[:], AT.Tanh, bias=zs_slot[:]
                    )
                    psum_nt = pnt.tile([MB, D], F32, tag=f"pnt{b}")
                    nc.tensor.matmul(
                        psum_nt[:], nh[:], ident[:], is_transpose=True
                    )

                    # u' = G * nh^T + Ht ; ss = 0.5*colsumsq(u')
                    nc.vector.scalar_tensor_tensor(
                        up[b][:], psum_nt[:], G[:], Ht[:],
                        OP.mult, OP.add,
                    )
                    sq = lp.tile([MB, D], F32, tag=f"sq{b}")
                    ss = lp.tile([MB, 1], F32, tag=f"ss{b}")
                    nc.vector.scalar_tensor_tensor(
                        sq[:], up[b][:], 0.5, up[b][:],
                        OP.mult, OP.mult, accum_out=ss[:],
                    )
                    # rsqrt via magic seed on bits of x/2 + fused Newton
                    b1 = lp.tile([MB, 1], I32, tag=f"b1{b}")
                    nc.vector.tensor_scalar(
                        b1[:], ss[:].bitcast(I32), 1, None,
                        OP.logical_shift_right,
                    )
                    b2 = lp.tile([MB, 1], I32, tag=f"b2{b}")
                    nc.vector.tensor_scalar(
                        b2[:], b1[:], -1, MAGIC_HALF, OP.mult, OP.add
                    )
                    ya = lp.tile([MB, 1], F32, tag=f"ya{b}")
                    yb = lp.tile([MB, 1], F32, tag=f"yb{b}")
                    cur = b2[:].bitcast(F32)
                    for it in range(NEWTON_ITERS):
                        nc.vector.tensor_tensor(ya[:], ss[:], cur, OP.mult)
                        tgt = rs[b] if it == NEWTON_ITERS - 1 else yb
                        nc.vector._custom_dve(
                            RECIPROCAL_APPROX_NR,
                            out=tgt[:], in0=ya[:], in1=cur, s0=1.5,
                        )
                        cur = tgt[:]

                def step(t_idx):
                    s_slot = lp.tile([D, 1], F32, tag="s_slot")
                    nc.vector.tensor_copy(
                        s_slot[:], S_f[:, bass.ds(t_idx, 1)]
                    )
                    zs_slot = lp.tile([D, 1], F32, tag="zs_slot")
                    nc.vector.tensor_copy(
                        zs_slot[:], ZS[:, bass.ds(t_idx, 1)]
                    )
                    for b in range(NBLK):
                        block_step(t_idx, b, s_slot, zs_slot)

                if nf_steps >= UNROLL:
                    with tc.For_i(0, nf_steps, UNROLL,
                                  staggered_reset=STAGGERED) as i:
                        for u in range(UNROLL):
                            step(i + u)
                else:
                    for u in range(nf_steps):
                        step(u)

                # final normalize into natural layout
                for b in range(NBLK):
                    Ht = lp.tile([MB, D], F32, tag=f"Ht{b}")
                    nc.vector.tensor_scalar(
                        Ht[:], up[b][:], rs[b][:], None, OP.mult
                    )
                    psum_H = ph.tile([D, MB], F32, tag=f"pH{b}")
                    nc.tensor.matmul(
                        psum_H[:], Ht[:], ident[:], is_transpose=True
                    )
                    nc.vector.tensor_copy(
                        H_sb[:, b * MB : (b + 1) * MB], psum_H[:]
                    )

            nc.sync.dma_start(hout[:], H_sb[:])
            # ---- phase 5: readout ----
            with (
                tc.tile_pool(name="ro", bufs=1) as ro,
                tc.tile_pool(name="ps_ro", bufs=1, space="PSUM") as pro,
            ):
                psum_r = pro.tile([1, M_LOC], F32)
                nc.tensor.matmul(psum_r[:], q_col[:], H_sb[:])
                r_sb = ro.tile([1, M_LOC], F32)
                nc.vector.tensor_copy(r_sb[:], psum_r[:])

                rmax_l = ro.tile([1, 1], F32)
                nc.vector.tensor_reduce(rmax_l[:], r_sb[:], AX.X, OP.max)
                mx_in = dram.tile([1, 1], F32)
                mx_out = dram.tile([1, 1], F32)
                nc.sync.dma_start(mx_in[:], rmax_l[:])
                nc.gpsimd.collective_compute(
                    "AllReduce", OP.max, replica_groups=GROUP,
                    ins=[mx_in[:]], outs=[mx_out[:]],
                )
                rmax_g = ro.tile([1, 1], F32)
                nc.sync.dma_start(rmax_g[:], mx_out[:])
                nmax = ro.tile([1, 1], F32)
                nc.vector.tensor_scalar(
                    nmax[:], rmax_g[:], -1.0, None, OP.mult
                )

                e_loc = ro.tile([1, M_LOC], F32)
                nc.scalar.activation(e_loc[:], r_sb[:], AT.Exp, bias=nmax[:])
                sexp_l = ro.tile([1, 1], F32)
                nc.vector.tensor_reduce(sexp_l[:], e_loc[:], AX.X, OP.add)
                se_in = dram.tile([1, 1], F32)
                se_out = dram.tile([1, 1], F32)
                nc.sync.dma_start(se_in[:], sexp_l[:])
                nc.gpsimd.collective_compute(
                    "AllReduce", OP.add, replica_groups=GROUP,
                    ins=[se_in[:]], outs=[se_out[:]],
                )
                sexp_g = ro.tile([1, 1], F32)
                nc.sync.dma_start(sexp_g[:], se_out[:])
                rcp = ro.tile([1, 1], F32)
                nc.vector.reciprocal(rcp[:], sexp_g[:])
                p_loc = ro.tile([1, M_LOC], F32)
                nc.vector.tensor_scalar(
                    p_loc[:], e_loc[:], rcp[:], None, OP.mult
                )

                pb_ps = pro.tile([D, M_LOC], F32)
                nc.tensor.matmul(pb_ps[:], ones_row_f[:], p_loc[:])
                ph2 = ro.tile([D, M_LOC], F32)
                nc.vector.tensor_tensor(
                    ph2[:], H_sb[:], pb_ps[:], OP.mult
                )
                u_loc = ro.tile([D, 1], F32)
                nc.vector.tensor_reduce(u_loc[:], ph2[:], AX.X, OP.add)

                u_in = dram.tile([D, 1], F32)
                u_out = dram.tile([D, 1], F32)
                nc.sync.dma_start(u_in[:], u_loc[:])
                nc.gpsimd.collective_compute(
                    "AllReduce", OP.add, replica_groups=GROUP,
                    ins=[u_in[:]], outs=[u_out[:]],
                )
                u_g = ro.tile([D, 1], F32)
                nc.sync.dma_start(u_g[:], u_out[:])

                kt_sb = ro.tile([D, D], F32)
                nc.sync.dma_start(kt_sb[:], kt_in[:])
                ku_ps = pro.tile([D, 1], F32)
                nc.tensor.matmul(ku_ps[:], kt_sb[:], u_g[:])
                sig = ro.tile([D, 1], F32)
                nc.scalar.activation(
                    sig[:], ku_ps[:], AT.Sigmoid, bias=q_col[:]
                )
                rnt_sb = ro.tile([D, D], F32)
                nc.sync.dma_start(rnt_sb[:], rnt_in[:])
                ans_ps = pro.tile([D, 1], F32)
                nc.tensor.matmul(ans_ps[:], rnt_sb[:], sig[:])
                ans_sb = ro.tile([D, 1], F32)
                nc.vector.tensor_copy(ans_sb[:], ans_ps[:])
                nc.sync.dma_start(ans_out[:], ans_sb[:])

    nc.compile()
    return nc


def _get_nc(nf_steps):
    key = (nf_steps, UNROLL, NEWTON_ITERS, STAGGERED)
    if key not in _CACHE:
        _CACHE[key] = _build_nc(nf_steps)
    return _CACHE[key]


def _in_maps(E_s, Q, F, X, Y, Z, R, K, H0, W0):
    Rn = R / np.sqrt((R * R).sum(axis=1, keepdims=True))
    ident = np.eye(D, dtype=np.float32)
    common = {
        "f": F, "qq": Q,
        "xt": np.ascontiguousarray(X.T),
        "yt": np.ascontiguousarray(Y.T),
        "zt": np.ascontiguousarray(Z.T),
        "kt": np.ascontiguousarray(K.T),
        "rnt": np.ascontiguousarray(Rn.T),
        "ident": ident,
    }
    in_maps = []
    for c in range(N_CORES):
        m = dict(common)
        m["e"] = E_s[c * T_LOC : (c + 1) * T_LOC]
        h0c = H0[:, c * M_LOC : (c + 1) * M_LOC]
        m["h0t"] = np.ascontiguousarray(
            h0c.T.reshape(NBLK, MB, D)
        )
        m["w0"] = np.ascontiguousarray(W0[:, c * M_LOC : (c + 1) * M_LOC])
        in_maps.append(m)
    return in_maps


def kernel(E_s, Q, F, X, Y, Z, R, K, H0, W0, _nf_steps=NF, _trace=False):
    from concourse.bass_utils import run_bass_kernel_spmd

    E_s = np.ascontiguousarray(np.asarray(E_s, dtype=np.float32))
    Q = np.asarray(Q, dtype=np.float32)
    F = np.asarray(F, dtype=np.float32)
    X = np.asarray(X, dtype=np.float32)
    Y = np.asarray(Y, dtype=np.float32)
    Z = np.asarray(Z, dtype=np.float32)
    R = np.asarray(R, dtype=np.float32)
    K = np.asarray(K, dtype=np.float32)
    H0 = np.asarray(H0, dtype=np.float32)
    W0 = np.asarray(W0, dtype=np.float32)

    in_maps = _in_maps(E_s, Q, F, X, Y, Z, R, K, H0, W0)
    nc = _get_nc(_nf_steps)
    res = run_bass_kernel_spmd(
        nc, in_maps, list(range(N_CORES)), trace=_trace
    )
    out = res.results[0]["ans"].astype(np.float32)
    globals()["LAST_H"] = np.concatenate(
        [res.results[c]["hout"] for c in range(N_CORES)], axis=1
    )
    if _trace:
        kernel.last_exec_time_ns = res.exec_time_ns
    return out


kernel.last_exec_time_ns = None
